# revision 22
# baseline (speedup 1.0000x reference)
"""Fused attention kernel for Trainium2 (Bass/Tile), SPMD over 8 NeuronCores.

Problem: B=4, D=64, S=4096 fp32 attention
    A = softmax_k(K^T Q / sqrt(D));  R = V A;  out = concat(R, Q) on channel dim.

Sharding: 8 cores = 4 batches x 2 query-halves (Sq=2048 per core).

Structure (per core): j-outer loop over 32 k-tiles so the PE stationary
operand (k-tile for S = K^T Q, v-tile for R = V E) is loaded once per j and
reused across all q-columns -- 4x fewer LDWEIGHTS than q-outer.  Stage
(j, h) covers k-rows [128j, 128j+128) x q-cols [1024h, 1024h+1024).

exp is split across engines (ACT is 1 elem/cycle/lane and would otherwise be
the wall):
  - ACT stages: e = exp(0.125*s + ln(alpha)) via the activation LUT.
  - DVE stages: 2-phase exp2 bit trick.  y1 = round(s*A + B1) as int32 is a
    piecewise-linear approx of alpha1*2^(0.125*s*log2e) when bitcast to f32;
    y2 = y1 + 2^22 is the half-exponent-phase-shifted version.  GPSIMD adds
    the two bitcast-f32 values, averaging the ripple to ~+-1%.  The combined
    scale alpha = mean((f32(y1)+f32(y2)) / exp(t)) is matched on the ACT side
    via the free bias (ln alpha), so softmax normalization cancels it.
R consumes e as float32r (fp22-truncated fp32, 1 PE cycle/row at N>=512).
The softmax divide happens on the host: the kernel ships raw numerator+Z
(V gets a ones-row appended, so Z rides along as output row 64).
"""

import sys

sys.path.insert(0, "/opt/trn_rl_repo")

import numpy as np  # noqa: E402

B, D, S = 4, 64, 4096
NCORES = 8
SQ = S * B // NCORES  # 2048 queries per core
QT = 512              # q-tile width (PE moving operand / psum bank)
KT = 128              # k-tile width
NKT = S // KT         # 32 k-tiles
NQT = SQ // QT        # 4 q-tiles per core
NST = NKT * 2         # 64 stages: (j, h) with h = q-half of 1024 cols
VTW = D + 1           # v-tile width (ones row -> Z)

# DVE fast-exp constants.  t = (0.125*s - C) * log2e;  y1 = round(s*A + B1).
# C shifts all weights down so ACT-tile fp16 e-values stay under 65504
# (max logit 21.414 for this dataset); the shift cancels in softmax.
LOG2E = 1.4426950408889634
EXP_C = 11.5
TRICK_A = float(np.float32(0.125 * LOG2E * (1 << 23)))
TRICK_B1 = float(np.float32((127.0 - 0.0295 - EXP_C * LOG2E) * (1 << 23)))
# ln of mean((bitcast_f32(y1) + bitcast_f32(y2)) / exp(t)) over uniform t
ACT_BIAS = 0.900805 - EXP_C
# k-tiles handled by the DVE+GPSIMD path (both q-halves); rest on ACT
DVE_J = frozenset(j for j in range(NKT) if j % 3 == 1 and j >= 4)

_nc_cache = None


def _build():
    global _nc_cache
    if _nc_cache is not None:
        return _nc_cache
    import concourse.tile as tile
    from concourse import bacc, mybir

    nc = bacc.Bacc(None, target_bir_lowering=False)
    f32 = mybir.dt.float32
    f32r = mybir.dt.float32r
    f16 = mybir.dt.float16
    i32 = mybir.dt.int32
    Alu = mybir.AluOpType

    kst = nc.dram_tensor("kst", [2 * D, S], f16, kind="ExternalInput")
    qrep = nc.dram_tensor("qrep", [2 * D, SQ], f16, kind="ExternalInput")
    vtin = nc.dram_tensor("vtin", [KT, NKT * VTW], f32r, kind="ExternalInput")
    vtin16 = nc.dram_tensor("vtin16", [KT, NKT * VTW], f16, kind="ExternalInput")
    out_rz = nc.dram_tensor("out_rz", [VTW, SQ], f32, kind="ExternalOutput")

    with tile.TileContext(nc) as tc:
        with (
            tc.tile_pool(name="singles", bufs=1) as singles,
            tc.tile_pool(name="sb_e", bufs=13) as sb_e,
            tc.tile_pool(name="sb_y", bufs=6) as sb_y,
            tc.tile_pool(name="sb_o", bufs=1) as sb_o,
            tc.tile_pool(name="ps_s", bufs=2, space="PSUM") as ps_s,
            tc.tile_pool(name="ps_r", bufs=1, space="PSUM") as ps_r,
        ):
            k_sb = singles.tile([2 * D, S], f16)
            q_sb = singles.tile([2 * D, SQ], f16)
            vt_sb = singles.tile([KT, NKT * VTW], f32r)
            vt16_sb = singles.tile([KT, NKT * VTW], f16)
            wu_a = singles.tile([KT, 16], i32)
            wu_b = singles.tile([KT, 16], i32)
            bias_sb = singles.tile([KT, 1], f32)
            nc.vector.memset(bias_sb, ACT_BIAS)

            # Input DMAs, first-needed first.  sync carries k+q, gpsimd vt.
            nc.sync.dma_start(out=k_sb[:, :KT], in_=kst[:, :KT])
            nc.sync.dma_start(out=q_sb[:, :QT], in_=qrep[:, :QT])
            nc.sync.dma_start(out=q_sb[:, QT:], in_=qrep[:, QT:])
            nc.sync.dma_start(out=k_sb[:, KT:], in_=kst[:, KT:])
            nc.gpsimd.dma_start(out=vt16_sb[:, : 4 * VTW], in_=vtin16[:, : 4 * VTW])
            nc.gpsimd.dma_start(out=vt16_sb[:, 4 * VTW :], in_=vtin16[:, 4 * VTW :])
            nc.gpsimd.dma_start(out=vt_sb[:, :], in_=vtin[:, :])

            # GPSIMD warmup: force the tensor_tensor ucode IRAM load to happen
            # during the DMA head, not on the first DVE-stage add.
            nc.gpsimd.memset(wu_a, 1)
            nc.gpsimd.memset(wu_b, 2)
            nc.gpsimd.tensor_add(wu_a, wu_a, wu_b)

            from concourse.tile_rust import add_dep_helper

            vt = vt_sb.rearrange("p (j d) -> p j d", j=NKT)
            vt16 = vt16_sb.rearrange("p (j d) -> p j d", j=NKT)

            r_ps = [
                ps_r.tile([VTW, QT], f32, tag=f"r{t}", name=f"r_ps{t}")
                for t in range(NQT)
            ]

            stage_e = [None] * NST
            stage_s_last = [None] * NST

            def emit_s(s):
                j, h = s // 2, s % 2
                s_ps = ps_s.tile([KT, 2 * QT], f32, tag="s_ps")
                for i in range(2):
                    tq = 2 * h + i
                    mm = nc.tensor.matmul(
                        s_ps[:, i * QT : (i + 1) * QT],
                        k_sb[:, j * KT : (j + 1) * KT],
                        q_sb[:, tq * QT : (tq + 1) * QT],
                        start=True,
                        stop=True,
                    )
                stage_s_last[s] = mm
                if j in DVE_J:
                    e_sb = sb_e.tile([KT, 2 * QT], f32r, tag="e32")
                    y1 = sb_y.tile([KT, 2 * QT], i32, tag="y1")
                    nc.vector.tensor_scalar(
                        out=y1, in0=s_ps,
                        scalar1=TRICK_A, scalar2=TRICK_B1,
                        op0=Alu.mult, op1=Alu.add,
                    )
                    y2 = sb_y.tile([KT, 2 * QT], i32, tag="y2")
                    nc.vector.tensor_scalar_add(y2, y1, 1 << 22)
                    nc.gpsimd.tensor_add(e_sb, y1.bitcast(f32), y2.bitcast(f32))
                else:
                    e_sb = sb_e.tile([KT, 2 * QT], f16, tag="e16")
                    nc.scalar.activation(
                        out=e_sb,
                        in_=s_ps,
                        func=mybir.ActivationFunctionType.Exp,
                        scale=0.125,
                        bias=bias_sb,
                    )
                stage_e[s] = e_sb

            def emit_r(s, barrier=None):
                j, h = s // 2, s % 2
                e_t = stage_e[s]
                stage_e[s] = None
                vt_j = vt[:, j, :] if j in DVE_J else vt16[:, j, :]
                for i in range(2):
                    tq = 2 * h + i
                    mm = nc.tensor.matmul(
                        r_ps[tq],
                        vt_j,
                        e_t[:, i * QT : (i + 1) * QT],
                        start=(j == 0),
                        stop=(j == NKT - 1),
                    )
                    if barrier is not None:
                        # Pin R(s) behind S(s+LAG) in the PE queue so the
                        # tile scheduler cannot collapse the pipeline depth
                        # below the DVE->GPSIMD exp-chain latency.
                        add_dep_helper(
                            mm.ins, barrier.ins, sync=True,
                            reason="hold R-matmul back for exp-chain latency",
                        )

            # Pair-grouped emission: all 4 S-matmuls of k-tile j are adjacent
            # in the PE queue (one stationary load), then all 4 R-matmuls of
            # k-tile j-LAGJ (one vt load).
            LAGJ = 5
            for j in range(NKT):
                emit_s(2 * j)
                emit_s(2 * j + 1)
                if j >= LAGJ:
                    bar = stage_s_last[2 * j + 1]
                    emit_r(2 * (j - LAGJ), barrier=bar)
                    emit_r(2 * (j - LAGJ) + 1, barrier=bar)
            for j in range(NKT - LAGJ, NKT):
                emit_r(2 * j)
                emit_r(2 * j + 1)

            rz_sb = sb_o.tile([VTW, SQ], f32, tag="rz")
            for tq in range(NQT):
                if tq % 2 == 0:
                    nc.scalar.copy(
                        out=rz_sb[:, tq * QT : (tq + 1) * QT], in_=r_ps[tq]
                    )
                else:
                    nc.vector.tensor_copy(
                        out=rz_sb[:, tq * QT : (tq + 1) * QT], in_=r_ps[tq]
                    )
                if tq == 1:
                    nc.sync.dma_start(
                        out=out_rz[:, : 2 * QT], in_=rz_sb[:, : 2 * QT]
                    )
                if tq == 3:
                    nc.gpsimd.dma_start(
                        out=out_rz[:, 2 * QT :], in_=rz_sb[:, 2 * QT :]
                    )

    nc.compile()
    _nc_cache = nc
    return nc


def _in_maps(K, V, Q):
    K = np.asarray(K, dtype=np.float32)
    V = np.asarray(V, dtype=np.float32)
    Q = np.asarray(Q, dtype=np.float32)
    maps = []
    for c in range(NCORES):
        b, h = c // 2, c % 2
        khi = K[b].astype(np.float16)
        klo = (K[b] - khi.astype(np.float32)).astype(np.float16)
        kst = np.concatenate([khi, klo], axis=0)  # [128, S]
        qhi = Q[b, :, h * SQ : (h + 1) * SQ].astype(np.float16)
        qrep = np.concatenate([qhi, qhi], axis=0)  # [128, SQ]
        # v-tiles: vt[p, j, d] = V[b, d, KT*j + p]; vt[p, j, D] = 1.0
        vt = np.empty((KT, NKT, VTW), dtype=np.float32)
        vt[:, :, :D] = V[b].T.reshape(NKT, KT, D).transpose(1, 0, 2)
        vt[:, :, D] = 1.0
        vt = vt.reshape(KT, NKT * VTW)
        maps.append(
            {
                "kst": np.ascontiguousarray(kst),
                "qrep": np.ascontiguousarray(qrep),
                "vtin": np.ascontiguousarray(vt),
                "vtin16": np.ascontiguousarray(vt.astype(np.float16)),
            }
        )
    return maps


def _run(K, V, Q, trace=False):
    from concourse.bass_utils import run_bass_kernel_spmd

    nc = _build()
    res = run_bass_kernel_spmd(
        nc, _in_maps(K, V, Q), list(range(NCORES)), trace=trace
    )
    Q = np.asarray(Q, dtype=np.float32)
    out = np.empty((B, 2 * D, S), dtype=np.float32)
    out[:, D : 2 * D, :] = Q
    for c in range(NCORES):
        b, h = c // 2, c % 2
        rz = res.results[c]["out_rz"].astype(np.float64)
        out[b, 0:D, h * SQ : (h + 1) * SQ] = (
            rz[0:D] / rz[D : D + 1]
        ).astype(np.float32)
    return out, res


def kernel(K, V, Q):
    out, _ = _run(K, V, Q, trace=False)
    return out


# revision 23
# speedup vs baseline: 1.2382x; 1.2382x over previous
"""Fused attention kernel for Trainium2 (Bass/Tile), SPMD over 8 NeuronCores.

Problem: B=4, D=64, S=4096 fp32 attention
    A = softmax_k(K^T Q / sqrt(D));  R = V A;  out = concat(R, Q) on channel dim.

Sharding: 8 cores = 4 batches x 2 query-halves (Sq=2048 per core).

Structure (per core): j-outer loop over 32 k-tiles so the PE stationary
operand (k-tile for S = K^T Q, v-tile for R = V E) is loaded once per j and
reused across all q-columns.  Stage (j, h) covers k-rows [128j, 128j+128)
x q-cols [1024h, 1024h+1024).

exp is split across engines (ACT is 1 elem/cycle/lane and would otherwise be
the wall):
  - ACT stages: e = exp(0.125*s + ln(alpha)) via the activation LUT.
  - DVE stages: 2-phase exp2 bit trick.  y1 = round(s*A + B1) as int32 is a
    piecewise-linear approx of alpha1*2^(0.125*s*log2e) when bitcast to f32;
    y2 = y1 + 2^22 is the half-exponent-phase-shifted version.  GPSIMD adds
    the two bitcast-f32 values, averaging the ripple to ~+-1%.  The combined
    scale alpha = mean((f32(y1)+f32(y2)) / exp(t)) is matched on the ACT side
    via the free bias (ln alpha), so softmax normalization cancels it.
R consumes e as float32r (fp22-truncated fp32, 1 PE cycle/row at N>=512).
The softmax divide happens on the host: the kernel ships raw numerator+Z
(V gets a ones-row appended, so Z rides along as output row 64).
"""

import sys

sys.path.insert(0, "/opt/trn_rl_repo")

import numpy as np  # noqa: E402

B, D, S = 4, 64, 4096
NCORES = 8
SQ = S * B // NCORES  # 2048 queries per core
QT = 512              # q-tile width (PE moving operand / psum bank)
KT = 128              # k-tile width
NKT = S // KT         # 32 k-tiles
NQT = SQ // QT        # 4 q-tiles per core
NST = NKT * 2         # 64 stages: (j, h) with h = q-half of 1024 cols
VTW = D + 1           # v-tile width (ones row -> Z)

# DVE fast-exp constants.  t = 0.125*s*log2e;  y1 = round(s*A + B1) int32.
LOG2E = 1.4426950408889634
TRICK_A = float(np.float32(0.125 * LOG2E * (1 << 23)))
TRICK_B1 = float(np.float32((127.0 - 0.0295) * (1 << 23)))
# ln of mean((bitcast_f32(y1) + bitcast_f32(y2)) / exp(t)) over uniform t
ACT_BIAS = 0.900805
# stages handled by the DVE+GPSIMD path (rest on ACT); s % 3 == 1 -> 21/64
DVE_STAGE = [s % 3 == 1 for s in range(NST)]

_nc_cache = None


def _build():
    global _nc_cache
    if _nc_cache is not None:
        return _nc_cache
    import concourse.tile as tile
    from concourse import bacc, mybir

    nc = bacc.Bacc(None, target_bir_lowering=False)
    f32 = mybir.dt.float32
    f32r = mybir.dt.float32r
    f16 = mybir.dt.float16
    i32 = mybir.dt.int32
    Alu = mybir.AluOpType

    kst = nc.dram_tensor("kst", [2 * D, S], f16, kind="ExternalInput")
    qrep = nc.dram_tensor("qrep", [2 * D, SQ], f16, kind="ExternalInput")
    vtin = nc.dram_tensor("vtin", [KT, NKT * VTW], f32r, kind="ExternalInput")
    out_rz = nc.dram_tensor("out_rz", [VTW, SQ], f32, kind="ExternalOutput")

    with tile.TileContext(nc) as tc:
        with (
            tc.tile_pool(name="singles", bufs=1) as singles,
            tc.tile_pool(name="sb_e", bufs=9) as sb_e,
            tc.tile_pool(name="sb_y", bufs=4) as sb_y,
            tc.tile_pool(name="sb_o", bufs=1) as sb_o,
            tc.tile_pool(name="ps_s", bufs=2, space="PSUM") as ps_s,
            tc.tile_pool(name="ps_r", bufs=1, space="PSUM") as ps_r,
        ):
            k_sb = singles.tile([2 * D, S], f16)
            q_sb = singles.tile([2 * D, SQ], f16)
            vt_sb = singles.tile([KT, NKT * VTW], f32r)
            wu_a = singles.tile([KT, 16], i32)
            wu_b = singles.tile([KT, 16], i32)
            bias_sb = singles.tile([KT, 1], f32)
            nc.vector.memset(bias_sb, ACT_BIAS)

            # Input DMAs, first-needed first.  sync carries k+q, gpsimd vt.
            nc.sync.dma_start(out=k_sb[:, :KT], in_=kst[:, :KT])
            nc.sync.dma_start(out=q_sb[:, :QT], in_=qrep[:, :QT])
            nc.sync.dma_start(out=q_sb[:, QT:], in_=qrep[:, QT:])
            nc.sync.dma_start(out=k_sb[:, KT:], in_=kst[:, KT:])
            nc.gpsimd.dma_start(out=vt_sb[:, : 4 * VTW], in_=vtin[:, : 4 * VTW])
            nc.gpsimd.dma_start(out=vt_sb[:, 4 * VTW :], in_=vtin[:, 4 * VTW :])

            # GPSIMD warmup: force the tensor_tensor ucode IRAM load to happen
            # during the DMA head, not on the first DVE-stage add.
            nc.gpsimd.memset(wu_a, 1)
            nc.gpsimd.memset(wu_b, 2)
            nc.gpsimd.tensor_add(wu_a, wu_a, wu_b)

            from concourse.tile_rust import add_dep_helper

            vt = vt_sb.rearrange("p (j d) -> p j d", j=NKT)

            r_ps = [
                ps_r.tile([VTW, QT], f32, tag=f"r{t}", name=f"r_ps{t}")
                for t in range(NQT)
            ]

            stage_e = [None] * NST
            stage_s_last = [None] * NST

            def emit_s(s):
                j, h = s // 2, s % 2
                s_ps = ps_s.tile([KT, 2 * QT], f32, tag="s_ps")
                for i in range(2):
                    tq = 2 * h + i
                    mm = nc.tensor.matmul(
                        s_ps[:, i * QT : (i + 1) * QT],
                        k_sb[:, j * KT : (j + 1) * KT],
                        q_sb[:, tq * QT : (tq + 1) * QT],
                        start=True,
                        stop=True,
                    )
                stage_s_last[s] = mm
                e_sb = sb_e.tile([KT, 2 * QT], f32r, tag="e_sb")
                if DVE_STAGE[s]:
                    y1 = sb_y.tile([KT, 2 * QT], i32, tag="y1")
                    nc.vector.tensor_scalar(
                        out=y1, in0=s_ps,
                        scalar1=TRICK_A, scalar2=TRICK_B1,
                        op0=Alu.mult, op1=Alu.add,
                    )
                    y2 = sb_y.tile([KT, 2 * QT], i32, tag="y2")
                    nc.vector.tensor_scalar_add(y2, y1, 1 << 22)
                    nc.gpsimd.tensor_add(e_sb, y1.bitcast(f32), y2.bitcast(f32))
                else:
                    nc.scalar.activation(
                        out=e_sb,
                        in_=s_ps,
                        func=mybir.ActivationFunctionType.Exp,
                        scale=0.125,
                        bias=bias_sb,
                    )
                stage_e[s] = e_sb

            def emit_r(s, barrier=None):
                j, h = s // 2, s % 2
                e32r = stage_e[s]
                stage_e[s] = None
                for i in range(2):
                    tq = 2 * h + i
                    mm = nc.tensor.matmul(
                        r_ps[tq],
                        vt[:, j, :],
                        e32r[:, i * QT : (i + 1) * QT],
                        start=(j == 0),
                        stop=(j == NKT - 1),
                    )
                    if barrier is not None:
                        # Pin R(s) behind S(s+LAG) in the PE queue so the
                        # tile scheduler cannot collapse the pipeline depth
                        # below the DVE->GPSIMD exp-chain latency.
                        add_dep_helper(
                            mm.ins, barrier.ins, sync=True,
                            reason="hold R-matmul back for exp-chain latency",
                        )

            # Pair-grouped emission: all 4 S-matmuls of k-tile j are adjacent
            # in the PE queue (one stationary load), then all 4 R-matmuls of
            # k-tile j-LAGJ (one vt load).
            LAGJ = 3
            for j in range(NKT):
                emit_s(2 * j)
                emit_s(2 * j + 1)
                if j >= LAGJ:
                    bar = stage_s_last[2 * j + 1]
                    emit_r(2 * (j - LAGJ), barrier=bar)
                    emit_r(2 * (j - LAGJ) + 1, barrier=bar)
            for j in range(NKT - LAGJ, NKT):
                emit_r(2 * j)
                emit_r(2 * j + 1)

            rz_sb = sb_o.tile([VTW, SQ], f32, tag="rz")
            for tq in range(NQT):
                if tq % 2 == 0:
                    nc.scalar.copy(
                        out=rz_sb[:, tq * QT : (tq + 1) * QT], in_=r_ps[tq]
                    )
                else:
                    nc.vector.tensor_copy(
                        out=rz_sb[:, tq * QT : (tq + 1) * QT], in_=r_ps[tq]
                    )
                if tq == 1:
                    nc.sync.dma_start(
                        out=out_rz[:, : 2 * QT], in_=rz_sb[:, : 2 * QT]
                    )
                if tq == 3:
                    nc.gpsimd.dma_start(
                        out=out_rz[:, 2 * QT :], in_=rz_sb[:, 2 * QT :]
                    )

    nc.compile()
    _nc_cache = nc
    return nc


def _in_maps(K, V, Q):
    K = np.asarray(K, dtype=np.float32)
    V = np.asarray(V, dtype=np.float32)
    Q = np.asarray(Q, dtype=np.float32)
    maps = []
    for c in range(NCORES):
        b, h = c // 2, c % 2
        khi = K[b].astype(np.float16)
        klo = (K[b] - khi.astype(np.float32)).astype(np.float16)
        kst = np.concatenate([khi, klo], axis=0)  # [128, S]
        qhi = Q[b, :, h * SQ : (h + 1) * SQ].astype(np.float16)
        qrep = np.concatenate([qhi, qhi], axis=0)  # [128, SQ]
        # v-tiles: vt[p, j, d] = V[b, d, KT*j + p]; vt[p, j, D] = 1.0
        vt = np.empty((KT, NKT, VTW), dtype=np.float32)
        vt[:, :, :D] = V[b].T.reshape(NKT, KT, D).transpose(1, 0, 2)
        vt[:, :, D] = 1.0
        maps.append(
            {
                "kst": np.ascontiguousarray(kst),
                "qrep": np.ascontiguousarray(qrep),
                "vtin": np.ascontiguousarray(vt.reshape(KT, NKT * VTW)),
            }
        )
    return maps


def _run(K, V, Q, trace=False):
    from concourse.bass_utils import run_bass_kernel_spmd

    nc = _build()
    res = run_bass_kernel_spmd(
        nc, _in_maps(K, V, Q), list(range(NCORES)), trace=trace
    )
    Q = np.asarray(Q, dtype=np.float32)
    out = np.empty((B, 2 * D, S), dtype=np.float32)
    out[:, D : 2 * D, :] = Q
    for c in range(NCORES):
        b, h = c // 2, c % 2
        rz = res.results[c]["out_rz"].astype(np.float64)
        out[b, 0:D, h * SQ : (h + 1) * SQ] = (
            rz[0:D] / rz[D : D + 1]
        ).astype(np.float32)
    return out, res


def kernel(K, V, Q):
    out, _ = _run(K, V, Q, trace=False)
    return out
